# revision 24
# baseline (speedup 1.0000x reference)
"""LoRA QKV fused projection kernel for 8 TRN2 NeuronCores.

Reference computation (T=8192 tokens, HID=4096, D=6144 out, S=8 slots, R=16):
    y = x @ W.T
    a[t,s,i,r] = sum_h x[t,h] * lora_A[s,i,r,h]         (down-proj, all slots)
    a *= onehot(token_to_slot)[t,s] * scaling[s]         (routing gate)
    d[t, :] = concat_i( sum_{s,r} a[t,s,i,r] * B_i[s,:,r] )   (up-proj)
    out = y + d
Sharding: data-parallel over tokens; core c owns tokens [c*1024, (c+1)*1024).

Per-core dataflow:
  * Phase A (LoRA down-proj aT = A @ x) in fp8 e4m3 DoubleRow over all 16
    k-tile PAIRS, one token half at a time (3 psum banks), j-major, paced
    by the pax tables streaming on the scalar ring. pax is split into
    pax1 = [A targets | x8 half0] and pax2 = [x8 half1] so half0 can
    start early.
  * Warmup is DMA-bound (~14MB of w0/w1, x, pax, gate on two rings while
    the PE has only Phase A + mb0 + mb1 to chew). A greedy list-scheduler
    emits the PE unit (A j-sweep bite or mb0/mb1 k-bite) whose modeled
    DMA arrival is earliest; banks: 3 A + 2+2 = 7 of 8. mb0/mb1's fp8
    pairs + B-matmuls run right after the gate. (gpsimd as a third DMA
    ring measured WORSE: its DRAIN ops add ~14us of idle.)
  * Phase B: routing gate (host-built onehot*scaling, expanded over rank)
    applied on DVE per half: ag = psum_a * gate, written bf16.
  * Phase C per output row-block mb (48), h-sequential per half: 24 bf16
    k-tiles (W pre-scaled by SP=2^15 to match the fp8 psum scale) + 4 fp8
    DoubleRow k-tile PAIRS into 1 psum bank; the LoRA up-proj B[mb] @
    ag[i] accumulates INTO THE SAME psum (start=False), fusing base +
    delta. psum->sbuf copy descales by 1/SP exactly, then one DMA out
    per half, alternating sync/scalar rings; half0's copy+DMA overlap
    half1's chain. Last two mbs' outputs split across both rings.

Precision (gate 2e-2): bf16 main + fp8 on 8/32 k-tiles + fp8 LoRA delta
measured 1.9206e-2 on the harness inputs (numpy sim matches HW to ~1e-6;
NP8=5 would be 2.13e-2 -- over the gate). fp8 e4m3 DoubleRow measured
~234ns per 512-col pair vs 2x218ns bf16 (1.86x). Measured ~650us HW exec
(baseline 677us), PE busy ~628us of 656us span (95.8%) -- the compute
roofline for this precision mix: matmul cost is column-count-bound, so
only a higher fp8 fraction could cut PE work, and the error budget caps
it at 4 pairs.
"""

import numpy as np
import ml_dtypes

# problem shape (hardcoded per harness contract)
T = 8192
HID = 4096
Q_SIZE = 4096
KV_SIZE = 1024
D = Q_SIZE + 2 * KV_SIZE  # 6144
S = 8
R = 16
NCORES = 8
P = 128

TC = T // NCORES          # 1024 tokens per core
MB = D // P               # 48 output row-blocks of 128
KA = HID // P             # 32 k-tiles
NP8 = 4                   # k-tile PAIRS of the main GEMM done in fp8 DoubleRow
KB = KA - 2 * NP8         # 24 k-tiles of the main GEMM done in bf16
NH = TC // 512            # 2 token halves (psum bank = 512 fp32)
JA = KA // 2              # 16 k-tile pairs
SP = np.float32(32768.0)  # psum scale SX*SA: bf16 W is pre-scaled by SP so
                          # fp8 (x*32)(W*1024) products accumulate coherently;
                          # the final psum->sbuf copy descales by 1/SP (exact)

_CACHE = {}


def _build_nc():
    import concourse.mybir as mybir
    import concourse.tile as tile
    from concourse import bacc

    bf16 = mybir.dt.bfloat16
    f32 = mybir.dt.float32
    f8 = mybir.dt.float8e4
    DR = mybir.MatmulPerfMode.DoubleRow

    nc = bacc.Bacc(None, target_bir_lowering=False, debug=False)

    # ---- DRAM parameters (per-core shapes)
    PX1 = 3 * P + 512  # pax1 row: 3 A targets then x8 half0
    x_d = nc.declare_dram_parameter("x_sh", [P, KB, TC], bf16, isOutput=False)
    w_d = nc.declare_dram_parameter("w_t", [MB, P, KB, P], bf16, isOutput=False)
    w8_d = nc.declare_dram_parameter("w8_t", [MB, P, NP8, 2, P], f8, isOutput=False)
    px1_d = nc.declare_dram_parameter("pax1", [P, JA, 2, PX1], f8, isOutput=False)
    px2_d = nc.declare_dram_parameter("pax2", [P, JA, 2, 512], f8, isOutput=False)
    b_d = nc.declare_dram_parameter("b_t", [P, MB, P], bf16, isOutput=False)
    g_d = nc.declare_dram_parameter("gate", [P, TC], bf16, isOutput=False)
    y_d = nc.declare_dram_parameter("y_out", [MB, P, TC], f32, isOutput=True)

    with tile.TileContext(nc) as tc:
        with tc.tile_pool(name="xres", bufs=1) as xres_pool, \
             tc.tile_pool(name="wp", bufs=3) as w_pool, \
             tc.tile_pool(name="ab", bufs=1) as ab_pool, \
             tc.tile_pool(name="agp", bufs=1) as ag_pool, \
             tc.tile_pool(name="stp", bufs=3) as st_pool, \
             tc.tile_pool(name="psum", bufs=8, space="PSUM") as ps_pool:

            # resident operands
            x_res = xres_pool.tile([P, KB, TC], bf16, tag="xres")
            px1_t = ab_pool.tile([P, JA, 2, PX1], f8, tag="pax1")
            px2_t = ab_pool.tile([P, JA, 2, 512], f8, tag="pax2")
            b_t = ab_pool.tile([P, MB, P], bf16, tag="b")
            gate_t = ab_pool.tile([P, TC], bf16, tag="gate")

            # Warmup is DMA-bound: ~14MB (w0/w1, x, pax, gate) streams on two
            # rings while the PE has only Phase A + mb0 + mb1 (~51us) to chew.
            # Model each ring's arrival times and emit DMAs/PE work in a
            # greedy arrival-matched order.
            BASE, BW = 4500.0, 0.117  # ring startup ns, bytes/ns per ring
            t_sc, t_sy = BASE, BASE
            rdy = {}

            def arr_sy(key, nbytes):
                nonlocal t_sy
                t_sy += nbytes / BW
                rdy[key] = t_sy

            def arr_sc(key, nbytes):
                nonlocal t_sc
                t_sc += nbytes / BW
                rdy[key] = t_sc

            # ---- sync ring: 1-ktile x bites, gate early, b (split)
            XKB = 128 * TC * 2  # bytes per x k-tile
            for k in range(0, 10):
                nc.sync.dma_start(out=x_res[:, k:k + 1, :], in_=x_d[:, k:k + 1, :])
                arr_sy(("x", k), XKB)
            nc.sync.dma_start(out=gate_t[:], in_=g_d[:])
            arr_sy("gate", TC * 2 * 128)
            for k in range(10, 18):
                nc.sync.dma_start(out=x_res[:, k:k + 1, :], in_=x_d[:, k:k + 1, :])
                arr_sy(("x", k), XKB)
            for (lo, hi) in [(18, 21), (21, KB)]:
                nc.sync.dma_start(out=x_res[:, lo:hi, :], in_=x_d[:, lo:hi, :])
                arr_sy(("x", lo), (hi - lo) * XKB)
                for k in range(lo, hi):
                    rdy[("x", k)] = rdy[("x", lo)]
            nc.sync.dma_start(out=b_t[:, 0:8], in_=b_d[:, 0:8])
            nc.sync.dma_start(out=b_t[:, 8:MB], in_=b_d[:, 8:MB])

            def load_w(mb, queue):
                w_t = w_pool.tile([P, KB, P], bf16, tag="w", name=f"w{mb}")
                queue.dma_start(out=w_t[:], in_=w_d[mb])
                w8_t = w_pool.tile([P, NP8, 2, P], f8, tag="w8", name=f"w8{mb}")
                queue.dma_start(out=w8_t[:], in_=w8_d[mb])
                return w_t, w8_t

            # ---- scalar ring: w0/w1 pieces, pax1/pax2 chunks interleaved
            WKB = 128 * P * 2       # bytes per w k-tile
            PX1B = 128 * 2 * PX1    # bytes per pax1 j
            PX2B = 128 * 2 * 512    # bytes per pax2 j
            W8B = 128 * NP8 * 2 * P

            w0_t = w_pool.tile([P, KB, P], bf16, tag="w", name="w0")
            w1_t = w_pool.tile([P, KB, P], bf16, tag="w", name="w1")
            w80_t = w_pool.tile([P, NP8, 2, P], f8, tag="w8", name="w80")
            w81_t = w_pool.tile([P, NP8, 2, P], f8, tag="w8", name="w81")

            def w_piece(w_t, wmb, lo, hi, key):
                nc.scalar.dma_start(out=w_t[:, lo:hi, :], in_=w_d[wmb, :, lo:hi, :])
                arr_sc((key, lo), (hi - lo) * WKB)
                for k in range(lo, hi):
                    rdy[(key, k)] = rdy[(key, lo)]

            def px1_piece(lo, hi):
                nc.scalar.dma_start(out=px1_t[:, lo:hi], in_=px1_d[:, lo:hi])
                arr_sc(("p1", lo), (hi - lo) * PX1B)
                for j in range(lo, hi):
                    rdy[("p1", j)] = rdy[("p1", lo)]

            def px2_piece(lo, hi):
                nc.scalar.dma_start(out=px2_t[:, lo:hi], in_=px2_d[:, lo:hi])
                arr_sc(("p2", lo), (hi - lo) * PX2B)
                for j in range(lo, hi):
                    rdy[("p2", j)] = rdy[("p2", lo)]

            w_piece(w0_t, 0, 0, 2, "w0")
            px1_piece(0, 1)
            px1_piece(1, 2)
            w_piece(w0_t, 0, 2, 8, "w0")
            px1_piece(2, 4)
            px1_piece(4, 6)
            w_piece(w0_t, 0, 8, 16, "w0")
            px1_piece(6, 8)
            w_piece(w1_t, 1, 0, 6, "w1")
            w_piece(w0_t, 0, 16, KB, "w0")
            nc.scalar.dma_start(out=w80_t[:], in_=w8_d[0])
            arr_sc("w80", W8B)
            px1_piece(8, 10)
            px1_piece(10, 12)
            w_piece(w1_t, 1, 6, 12, "w1")
            px1_piece(12, 14)
            px1_piece(14, JA)
            px2_piece(0, 4)
            w_piece(w1_t, 1, 12, KB, "w1")
            nc.scalar.dma_start(out=w81_t[:], in_=w8_d[1])
            arr_sc("w81", W8B)
            px2_piece(4, 8)
            px2_piece(8, 12)
            px2_piece(12, JA)

            # ---------------- PE emission ------------------------------------
            # Phase A current-half psums (3 banks) + mb0/mb1 (4) = 7 of 8.
            ps_a = [ps_pool.tile([P, 512], f32, tag="ps", name=f"ps_a{i}")
                    for i in range(3)]
            ps0 = [ps_pool.tile([P, 512], f32, tag="ps", name=f"pm0_{h}")
                   for h in range(NH)]
            ps1 = [ps_pool.tile([P, 512], f32, tag="ps", name=f"pm1_{h}")
                   for h in range(NH)]

            def a_jgroup(ps3, jlo, jhi, h):
                mv = (lambda j: px1_t[:, j, :, 3 * P:]) if h == 0 else \
                     (lambda j: px2_t[:, j, :, :])
                for j in range(jlo, jhi):
                    for i in range(3):
                        nc.tensor.matmul(
                            ps3[i][:],
                            px1_t[:, j, :, i * P:(i + 1) * P],
                            mv(j),
                            start=(j == 0), stop=(j == JA - 1),
                            perf_mode=DR,
                        )

            def mb_ksub(pss, w_t, klo, khi):
                for k in range(klo, khi):
                    for h in range(NH):
                        nc.tensor.matmul(
                            pss[h][:],
                            w_t[:, k, :],
                            x_res[:, k, h * 512:(h + 1) * 512],
                            start=(k == 0), stop=False,
                        )

            def mb_fp8(pss, w8_t):
                for jp in range(NP8):
                    j = KB // 2 + jp
                    for h in range(NH):
                        mv = px1_t[:, j, :, 3 * P:] if h == 0 else px2_t[:, j, :, :]
                        nc.tensor.matmul(
                            pss[h][:],
                            w8_t[:, jp, :, :],
                            mv,
                            start=False, stop=False,
                            perf_mode=DR,
                        )

            ag = [ag_pool.tile([P, TC], bf16, tag=f"ag{i}", name=f"ag{i}")
                  for i in range(3)]

            def gate_half(ps3, h):
                sl = slice(h * 512, (h + 1) * 512)
                for i in range(3):
                    nc.vector.tensor_mul(ag[i][:, sl], ps3[i][:], gate_t[:, sl])

            # Greedy warmup: emit the PE work unit whose operands arrive
            # earliest, tracking modeled PE time. Units: A0[j]/A1[j] (3 DR
            # mms each), M0[k]/M1[k] (2 bf16 mms each). A1 only after gate0
            # (psum bank reuse).
            MM_BF, MM_DR = 440.0, 715.0  # modeled 2x bf16 / 3x DR unit ns
            ps_a1 = [None, None, None]
            streams = {
                "A0": list(range(JA)), "A1": list(range(JA)),
                "M0": list(range(KB)), "M1": list(range(KB)),
            }
            gate0_done = False
            pe_t = BASE

            def unit_ready(s):
                if not streams[s]:
                    return None
                h = streams[s][0]
                if s == "A0":
                    return rdy[("p1", h)]
                if s == "A1":
                    if not gate0_done:
                        return None
                    return rdy[("p2", h)]
                if s == "M0":
                    return max(rdy[("w0", h)], rdy[("x", h)])
                return max(rdy[("w1", h)], rdy[("x", h)])

            while any(streams.values()):
                cand = [(unit_ready(s), s) for s in streams if unit_ready(s) is not None]
                t_r, s = min(cand)
                u = streams[s].pop(0)
                pe_t = max(pe_t, t_r)
                if s == "A0":
                    a_jgroup(ps_a, u, u + 1, 0)
                    pe_t += MM_DR
                    if u == JA - 1:
                        gate_half(ps_a, 0)
                        gate0_done = True
                        ps_a1[:] = [
                            ps_pool.tile([P, 512], f32, tag="ps", name=f"ps_b{i}")
                            for i in range(3)]
                elif s == "A1":
                    a_jgroup(ps_a1, u, u + 1, 1)
                    pe_t += MM_DR
                    if u == JA - 1:
                        gate_half(ps_a1, 1)
                elif s == "M0":
                    mb_ksub(ps0, w0_t, u, u + 1)
                    pe_t += MM_BF
                else:
                    mb_ksub(ps1, w1_t, u, u + 1)
                    pe_t += MM_BF

            mb_fp8(ps0, w80_t)
            mb_fp8(ps1, w81_t)

            def finish_mb(mb, pss, h_list=None):
                i = 0 if mb < Q_SIZE // P else (1 if mb < (Q_SIZE + KV_SIZE) // P else 2)
                st = st_pool.tile([P, TC], f32, tag="st", name=f"st{mb}")
                oq = nc.sync if mb % 2 == 0 else nc.scalar
                for h in (h_list if h_list is not None else range(NH)):
                    nc.tensor.matmul(
                        pss[h][:],
                        b_t[:, mb, :],
                        ag[i][:, h * 512:(h + 1) * 512],
                        start=False, stop=True,
                    )
                    nc.vector.tensor_scalar_mul(
                        st[:, h * 512:(h + 1) * 512], pss[h][:],
                        float(1.0 / SP))
                    oq.dma_start(
                        out=y_d[mb, :, h * 512:(h + 1) * 512],
                        in_=st[:, h * 512:(h + 1) * 512],
                    )
                return st

            finish_mb(0, ps0)
            finish_mb(1, ps1)

            # ------------- Phase C: remaining mb chains ---------------------
            # h-sequential: one psum bank per half, half0's B/copy/DMA overlap
            # half1's chain; last mb's final DMAs split to shorten the tail.
            for mb in range(2, MB):
                w_t, w8_t = load_w(mb, nc.scalar)
                i = 0 if mb < Q_SIZE // P else (1 if mb < (Q_SIZE + KV_SIZE) // P else 2)
                st = st_pool.tile([P, TC], f32, tag="st", name=f"st{mb}")
                oq = nc.sync if mb % 2 == 0 else nc.scalar
                pieces = 2 if mb >= MB - 2 else 1
                for h in range(NH):
                    ps = ps_pool.tile([P, 512], f32, tag="ps", name=f"pm{mb}_{h}")
                    for k in range(KB):
                        nc.tensor.matmul(
                            ps[:], w_t[:, k, :],
                            x_res[:, k, h * 512:(h + 1) * 512],
                            start=(k == 0), stop=False,
                        )
                    for jp in range(NP8):
                        j = KB // 2 + jp
                        mv = px1_t[:, j, :, 3 * P:] if h == 0 else px2_t[:, j, :, :]
                        nc.tensor.matmul(
                            ps[:], w8_t[:, jp, :, :], mv,
                            start=False, stop=False, perf_mode=DR,
                        )
                    nc.tensor.matmul(
                        ps[:], b_t[:, mb, :],
                        ag[i][:, h * 512:(h + 1) * 512],
                        start=False, stop=True,
                    )
                    nc.vector.tensor_scalar_mul(
                        st[:, h * 512:(h + 1) * 512], ps[:],
                        float(1.0 / SP))
                    pw = 512 // pieces
                    for pc in range(pieces):
                        lo = h * 512 + pc * pw
                        # final mbs: spread pieces over both rings
                        q = (nc.sync, nc.scalar)[pc % 2] if pieces > 1 else oq
                        q.dma_start(
                            out=y_d[mb, :, lo:lo + pw],
                            in_=st[:, lo:lo + pw],
                        )

    nc.compile()
    return nc


def _get_nc():
    if "nc" not in _CACHE:
        _CACHE["nc"] = _build_nc()
    return _CACHE["nc"]


def _prep_in_maps(x, W, lora_A, lora_B_q, lora_B_k, lora_B_v, scaling, token_to_slot):
    f = np.float32
    bf = ml_dtypes.bfloat16
    x = np.ascontiguousarray(x, dtype=f)
    W = np.ascontiguousarray(W, dtype=f)

    # x shard, moving operand: [c, p, ka, tl]  (h = ka*128 + p, t = c*1024 + tl)
    x_f32 = np.ascontiguousarray(
        x.reshape(NCORES, TC, KA, P).transpose(0, 3, 2, 1))
    x_sh = np.ascontiguousarray(x_f32[:, :, :KB, :]).astype(bf)
    # W stationary: [mb, p, ka, dl]  (d = mb*128 + dl)  -- replicated.
    # bf16 part pre-scaled by SP to match the fp8 psum scale; the last
    # 2*NP8 k-tiles go as fp8(W*1024) DoubleRow pairs.
    w_all = W.reshape(MB, P, KA, P).transpose(0, 3, 2, 1)
    w_t = np.ascontiguousarray(w_all[:, :, :KB, :] * SP).astype(bf)
    # fp8 e4m3 copies for the LoRA down-proj (DoubleRow pairs of k-tiles);
    # pax1 = [A targets | x8 half0], pax2 = [x8 half1]. 1/(SX*SA) descale
    # folds into the gate below.
    SX, SA = np.float32(32.0), np.float32(1024.0)
    f8 = ml_dtypes.float8_e4m3
    w8 = np.ascontiguousarray(
        (w_all[:, :, KB:, :] * SA).astype(f8).reshape(MB, P, NP8, 2, P))
    x8 = (x_f32 * SX).astype(f8).reshape(NCORES, P, JA, 2, TC)
    a_f32 = np.ascontiguousarray(
        np.asarray(lora_A, dtype=f).reshape(S, 3, R, KA, P).transpose(4, 3, 1, 0, 2)
        .reshape(P, KA, 3, S * R))
    a8 = (a_f32 * SA).astype(f8).reshape(P, JA, 2, 3 * S * R)
    pax1 = np.concatenate(
        [np.broadcast_to(a8, (NCORES,) + a8.shape), x8[..., 0:512]], axis=-1)
    pax1 = np.ascontiguousarray(pax1)
    pax2 = np.ascontiguousarray(x8[..., 512:])
    # LoRA B stationary: [(s r), mb, dl] -- replicated
    bq = np.asarray(lora_B_q, dtype=f).transpose(0, 2, 1).reshape(S * R, Q_SIZE)
    bk = np.asarray(lora_B_k, dtype=f).transpose(0, 2, 1).reshape(S * R, KV_SIZE)
    bv = np.asarray(lora_B_v, dtype=f).transpose(0, 2, 1).reshape(S * R, KV_SIZE)
    b_t = np.ascontiguousarray(
        np.concatenate([bq, bk, bv], axis=1).reshape(S * R, MB, P)).astype(bf)
    # routing gate, expanded over ranks: [c, (s r), tl]. The LoRA psum is
    # already SP x true scale (x*32 times A*1024), which matches the main
    # psum scale, so the gate is just the per-slot scaling.
    slot = np.asarray(token_to_slot).reshape(NCORES, TC)
    g = (slot[:, None, :] == np.arange(S, dtype=slot.dtype)[None, :, None])
    g = g.astype(f) * np.asarray(scaling, dtype=f)[None, :, None]
    gate = np.ascontiguousarray(np.repeat(g, R, axis=1)).astype(bf)

    in_maps = []
    for c in range(NCORES):
        in_maps.append({
            "x_sh": x_sh[c],
            "w_t": w_t,
            "w8_t": w8,
            "pax1": pax1[c],
            "pax2": pax2[c],
            "b_t": b_t,
            "gate": gate[c],
        })
    return in_maps


def _assemble(results):
    out = np.empty((T, D), dtype=np.float32)
    for c in range(NCORES):
        out[c * TC:(c + 1) * TC, :] = results[c]["y_out"].reshape(D, TC).T
    return out


def _run(inputs, trace=False):
    from concourse.bass_utils import run_bass_kernel_spmd
    nc = _get_nc()
    in_maps = _prep_in_maps(**inputs)
    res = run_bass_kernel_spmd(
        nc, in_maps, core_ids=list(range(NCORES)), trace=trace)
    return res


def kernel(**inputs) -> np.ndarray:
    res = _run(inputs, trace=False)
    return _assemble(res.results)


if __name__ == "__main__":
    rng = np.random.default_rng(0)
    ins = {
        "x": rng.standard_normal((T, HID)).astype(np.float32),
        "W": (rng.standard_normal((D, HID)) * 0.02).astype(np.float32),
        "lora_A": (rng.standard_normal((S, 3, R, HID)) * 0.02).astype(np.float32),
        "lora_B_q": (rng.standard_normal((S, Q_SIZE, R)) * 0.02).astype(np.float32),
        "lora_B_k": (rng.standard_normal((S, KV_SIZE, R)) * 0.02).astype(np.float32),
        "lora_B_v": (rng.standard_normal((S, KV_SIZE, R)) * 0.02).astype(np.float32),
        "scaling": rng.uniform(0.5, 2.0, S).astype(np.float32),
        "token_to_slot": rng.integers(0, S, T).astype(np.int32),
    }
    out = kernel(**ins)
    print("out", out.shape, out.dtype)


# revision 25
# speedup vs baseline: 1.0046x; 1.0046x over previous
"""LoRA QKV fused projection kernel for 8 TRN2 NeuronCores.

Reference computation (T=8192 tokens, HID=4096, D=6144 out, S=8 slots, R=16):
    y = x @ W.T
    a[t,s,i,r] = sum_h x[t,h] * lora_A[s,i,r,h]         (down-proj, all slots)
    a *= onehot(token_to_slot)[t,s] * scaling[s]         (routing gate)
    d[t, :] = concat_i( sum_{s,r} a[t,s,i,r] * B_i[s,:,r] )   (up-proj)
    out = y + d
Sharding: data-parallel over tokens; core c owns tokens [c*1024, (c+1)*1024).

Per-core dataflow:
  * Phase A (LoRA down-proj aT = A @ x) in fp8 e4m3 DoubleRow over all 16
    k-tile PAIRS, one token half at a time (3 psum banks), j-major, paced
    by the pax tables streaming on the scalar ring. pax is split into
    pax1 = [A targets | x8 half0] and pax2 = [x8 half1] so half0 can
    start early.
  * Warmup is DMA-bound (~14MB of w0/w1, x, pax, gate on two rings while
    the PE has only Phase A + mb0 + mb1 to chew). A greedy list-scheduler
    emits the PE unit (A j-sweep bite or mb0/mb1 k-bite) whose modeled
    DMA arrival is earliest; banks: 3 A + 2+2 = 7 of 8. mb0/mb1's fp8
    pairs + B-matmuls run right after the gate. (gpsimd as a third DMA
    ring measured WORSE: its DRAIN ops add ~14us of idle.)
  * Phase B: routing gate (host-built onehot*scaling, expanded over rank)
    applied on DVE per half: ag = psum_a * gate, written bf16.
  * Phase C per output row-block mb (48), h-sequential per half: 24 bf16
    k-tiles (W pre-scaled by SP=2^15 to match the fp8 psum scale) + 4 fp8
    DoubleRow k-tile PAIRS into 1 psum bank; the LoRA up-proj B[mb] @
    ag[i] accumulates INTO THE SAME psum (start=False), fusing base +
    delta. psum->sbuf copy descales by 1/SP exactly, then one DMA out
    per half, alternating sync/scalar rings; half0's copy+DMA overlap
    half1's chain. Last two mbs' outputs split across both rings.

Precision (gate 2e-2): bf16 main + fp8 on 8/32 k-tiles + fp8 LoRA delta
measured 1.9206e-2 on the harness inputs (numpy sim matches HW to ~1e-6;
NP8=5 would be 2.13e-2 -- over the gate). fp8 e4m3 DoubleRow measured
~234ns per 512-col pair vs 2x218ns bf16 (1.86x). Measured ~650us HW exec
(baseline 677us), PE busy ~628us of 656us span (95.8%) -- the compute
roofline for this precision mix: matmul cost is column-count-bound, so
only a higher fp8 fraction could cut PE work, and the error budget caps
it at 4 pairs.
"""

import numpy as np
import ml_dtypes

# problem shape (hardcoded per harness contract)
T = 8192
HID = 4096
Q_SIZE = 4096
KV_SIZE = 1024
D = Q_SIZE + 2 * KV_SIZE  # 6144
S = 8
R = 16
NCORES = 8
P = 128

TC = T // NCORES          # 1024 tokens per core
MB = D // P               # 48 output row-blocks of 128
KA = HID // P             # 32 k-tiles
NP8 = 4                   # k-tile PAIRS of the main GEMM done in fp8 DoubleRow
KB = KA - 2 * NP8         # 24 k-tiles of the main GEMM done in bf16
NH = TC // 512            # 2 token halves (psum bank = 512 fp32)
JA = KA // 2              # 16 k-tile pairs
SP = np.float32(32768.0)  # psum scale SX*SA: bf16 W is pre-scaled by SP so
                          # fp8 (x*32)(W*1024) products accumulate coherently;
                          # the final psum->sbuf copy descales by 1/SP (exact)

_CACHE = {}


def _build_nc():
    import concourse.mybir as mybir
    import concourse.tile as tile
    from concourse import bacc

    bf16 = mybir.dt.bfloat16
    f32 = mybir.dt.float32
    f8 = mybir.dt.float8e4
    DR = mybir.MatmulPerfMode.DoubleRow

    nc = bacc.Bacc(None, target_bir_lowering=False, debug=False)

    # ---- DRAM parameters (per-core shapes)
    PX1 = 3 * P + 512  # pax1 row: 3 A targets then x8 half0
    x_d = nc.declare_dram_parameter("x_sh", [P, KB, TC], bf16, isOutput=False)
    w_d = nc.declare_dram_parameter("w_t", [MB, P, KB, P], bf16, isOutput=False)
    w8_d = nc.declare_dram_parameter("w8_t", [MB, P, NP8, 2, P], f8, isOutput=False)
    px1_d = nc.declare_dram_parameter("pax1", [P, JA, 2, PX1], f8, isOutput=False)
    px2_d = nc.declare_dram_parameter("pax2", [P, JA, 2, 512], f8, isOutput=False)
    b_d = nc.declare_dram_parameter("b_t", [P, MB, P], bf16, isOutput=False)
    g_d = nc.declare_dram_parameter("gate", [P, TC], f32, isOutput=False)
    y_d = nc.declare_dram_parameter("y_out", [MB, P, TC], f32, isOutput=True)

    with tile.TileContext(nc) as tc:
        with tc.tile_pool(name="xres", bufs=1) as xres_pool, \
             tc.tile_pool(name="wp", bufs=3) as w_pool, \
             tc.tile_pool(name="ab", bufs=1) as ab_pool, \
             tc.tile_pool(name="agp", bufs=1) as ag_pool, \
             tc.tile_pool(name="stp", bufs=3) as st_pool, \
             tc.tile_pool(name="psum", bufs=8, space="PSUM") as ps_pool:

            # resident operands
            x_res = xres_pool.tile([P, KB, TC], bf16, tag="xres")
            px1_t = ab_pool.tile([P, JA, 2, PX1], f8, tag="pax1")
            px2_t = ab_pool.tile([P, JA, 2, 512], f8, tag="pax2")
            b_t = ab_pool.tile([P, MB, P], bf16, tag="b")
            gate_t = ab_pool.tile([P, TC], f32, tag="gate")

            # Warmup is DMA-bound: ~14MB (w0/w1, x, pax, gate) streams on two
            # rings while the PE has only Phase A + mb0 + mb1 (~51us) to chew.
            # Model each ring's arrival times and emit DMAs/PE work in a
            # greedy arrival-matched order.
            BASE, BW = 4500.0, 0.117  # ring startup ns, bytes/ns per ring
            t_sc, t_sy = BASE, BASE
            rdy = {}

            def arr_sy(key, nbytes):
                nonlocal t_sy
                t_sy += nbytes / BW
                rdy[key] = t_sy

            def arr_sc(key, nbytes):
                nonlocal t_sc
                t_sc += nbytes / BW
                rdy[key] = t_sc

            # ---- sync ring: 1-ktile x bites, gate early, b (split)
            XKB = 128 * TC * 2  # bytes per x k-tile
            for k in range(0, 10):
                nc.sync.dma_start(out=x_res[:, k:k + 1, :], in_=x_d[:, k:k + 1, :])
                arr_sy(("x", k), XKB)
            nc.sync.dma_start(out=gate_t[:], in_=g_d[:])
            arr_sy("gate", TC * 4 * 128)
            for k in range(10, 18):
                nc.sync.dma_start(out=x_res[:, k:k + 1, :], in_=x_d[:, k:k + 1, :])
                arr_sy(("x", k), XKB)
            for (lo, hi) in [(18, 21), (21, KB)]:
                nc.sync.dma_start(out=x_res[:, lo:hi, :], in_=x_d[:, lo:hi, :])
                arr_sy(("x", lo), (hi - lo) * XKB)
                for k in range(lo, hi):
                    rdy[("x", k)] = rdy[("x", lo)]
            nc.sync.dma_start(out=b_t[:, 0:8], in_=b_d[:, 0:8])
            nc.sync.dma_start(out=b_t[:, 8:MB], in_=b_d[:, 8:MB])

            def load_w(mb, queue):
                w_t = w_pool.tile([P, KB, P], bf16, tag="w", name=f"w{mb}")
                queue.dma_start(out=w_t[:], in_=w_d[mb])
                w8_t = w_pool.tile([P, NP8, 2, P], f8, tag="w8", name=f"w8{mb}")
                queue.dma_start(out=w8_t[:], in_=w8_d[mb])
                return w_t, w8_t

            # ---- scalar ring: w0/w1 pieces, pax1/pax2 chunks interleaved
            WKB = 128 * P * 2       # bytes per w k-tile
            PX1B = 128 * 2 * PX1    # bytes per pax1 j
            PX2B = 128 * 2 * 512    # bytes per pax2 j
            W8B = 128 * NP8 * 2 * P

            w0_t = w_pool.tile([P, KB, P], bf16, tag="w", name="w0")
            w1_t = w_pool.tile([P, KB, P], bf16, tag="w", name="w1")
            w80_t = w_pool.tile([P, NP8, 2, P], f8, tag="w8", name="w80")
            w81_t = w_pool.tile([P, NP8, 2, P], f8, tag="w8", name="w81")

            def w_piece(w_t, wmb, lo, hi, key):
                nc.scalar.dma_start(out=w_t[:, lo:hi, :], in_=w_d[wmb, :, lo:hi, :])
                arr_sc((key, lo), (hi - lo) * WKB)
                for k in range(lo, hi):
                    rdy[(key, k)] = rdy[(key, lo)]

            def px1_piece(lo, hi):
                nc.scalar.dma_start(out=px1_t[:, lo:hi], in_=px1_d[:, lo:hi])
                arr_sc(("p1", lo), (hi - lo) * PX1B)
                for j in range(lo, hi):
                    rdy[("p1", j)] = rdy[("p1", lo)]

            def px2_piece(lo, hi):
                nc.scalar.dma_start(out=px2_t[:, lo:hi], in_=px2_d[:, lo:hi])
                arr_sc(("p2", lo), (hi - lo) * PX2B)
                for j in range(lo, hi):
                    rdy[("p2", j)] = rdy[("p2", lo)]

            w_piece(w0_t, 0, 0, 2, "w0")
            px1_piece(0, 1)
            px1_piece(1, 2)
            w_piece(w0_t, 0, 2, 8, "w0")
            px1_piece(2, 4)
            px1_piece(4, 6)
            w_piece(w0_t, 0, 8, 16, "w0")
            px1_piece(6, 8)
            w_piece(w1_t, 1, 0, 6, "w1")
            w_piece(w0_t, 0, 16, KB, "w0")
            nc.scalar.dma_start(out=w80_t[:], in_=w8_d[0])
            arr_sc("w80", W8B)
            px1_piece(8, 10)
            px1_piece(10, 12)
            w_piece(w1_t, 1, 6, 12, "w1")
            px1_piece(12, 14)
            px1_piece(14, JA)
            px2_piece(0, 4)
            w_piece(w1_t, 1, 12, KB, "w1")
            nc.scalar.dma_start(out=w81_t[:], in_=w8_d[1])
            arr_sc("w81", W8B)
            px2_piece(4, 8)
            px2_piece(8, 12)
            px2_piece(12, JA)

            # ---------------- PE emission ------------------------------------
            # Phase A current-half psums (3 banks) + mb0/mb1 (4) = 7 of 8.
            ps_a = [ps_pool.tile([P, 512], f32, tag="ps", name=f"ps_a{i}")
                    for i in range(3)]
            ps0 = [ps_pool.tile([P, 512], f32, tag="ps", name=f"pm0_{h}")
                   for h in range(NH)]
            ps1 = [ps_pool.tile([P, 512], f32, tag="ps", name=f"pm1_{h}")
                   for h in range(NH)]

            def a_jgroup(ps3, jlo, jhi, h):
                mv = (lambda j: px1_t[:, j, :, 3 * P:]) if h == 0 else \
                     (lambda j: px2_t[:, j, :, :])
                for j in range(jlo, jhi):
                    for i in range(3):
                        nc.tensor.matmul(
                            ps3[i][:],
                            px1_t[:, j, :, i * P:(i + 1) * P],
                            mv(j),
                            start=(j == 0), stop=(j == JA - 1),
                            perf_mode=DR,
                        )

            def mb_ksub(pss, w_t, klo, khi):
                for k in range(klo, khi):
                    for h in range(NH):
                        nc.tensor.matmul(
                            pss[h][:],
                            w_t[:, k, :],
                            x_res[:, k, h * 512:(h + 1) * 512],
                            start=(k == 0), stop=False,
                        )

            def mb_fp8(pss, w8_t):
                for jp in range(NP8):
                    j = KB // 2 + jp
                    for h in range(NH):
                        mv = px1_t[:, j, :, 3 * P:] if h == 0 else px2_t[:, j, :, :]
                        nc.tensor.matmul(
                            pss[h][:],
                            w8_t[:, jp, :, :],
                            mv,
                            start=False, stop=False,
                            perf_mode=DR,
                        )

            ag = [ag_pool.tile([P, TC], bf16, tag=f"ag{i}", name=f"ag{i}")
                  for i in range(3)]

            def gate_half(ps3, h):
                sl = slice(h * 512, (h + 1) * 512)
                for i in range(3):
                    nc.vector.tensor_mul(ag[i][:, sl], ps3[i][:], gate_t[:, sl])

            # Greedy warmup: emit the PE work unit whose operands arrive
            # earliest, tracking modeled PE time. Units: A0[j]/A1[j] (3 DR
            # mms each), M0[k]/M1[k] (2 bf16 mms each). A1 only after gate0
            # (psum bank reuse).
            MM_BF, MM_DR = 440.0, 715.0  # modeled 2x bf16 / 3x DR unit ns
            ps_a1 = [None, None, None]
            streams = {
                "A0": list(range(JA)), "A1": list(range(JA)),
                "M0": list(range(KB)), "M1": list(range(KB)),
            }
            gate0_done = False
            pe_t = BASE

            def unit_ready(s):
                if not streams[s]:
                    return None
                h = streams[s][0]
                if s == "A0":
                    return rdy[("p1", h)]
                if s == "A1":
                    if not gate0_done:
                        return None
                    return rdy[("p2", h)]
                if s == "M0":
                    return max(rdy[("w0", h)], rdy[("x", h)])
                return max(rdy[("w1", h)], rdy[("x", h)])

            while any(streams.values()):
                cand = [(unit_ready(s), s) for s in streams if unit_ready(s) is not None]
                t_r, s = min(cand)
                u = streams[s].pop(0)
                pe_t = max(pe_t, t_r)
                if s == "A0":
                    a_jgroup(ps_a, u, u + 1, 0)
                    pe_t += MM_DR
                    if u == JA - 1:
                        gate_half(ps_a, 0)
                        gate0_done = True
                        ps_a1[:] = [
                            ps_pool.tile([P, 512], f32, tag="ps", name=f"ps_b{i}")
                            for i in range(3)]
                elif s == "A1":
                    a_jgroup(ps_a1, u, u + 1, 1)
                    pe_t += MM_DR
                    if u == JA - 1:
                        gate_half(ps_a1, 1)
                elif s == "M0":
                    mb_ksub(ps0, w0_t, u, u + 1)
                    pe_t += MM_BF
                else:
                    mb_ksub(ps1, w1_t, u, u + 1)
                    pe_t += MM_BF

            mb_fp8(ps0, w80_t)
            mb_fp8(ps1, w81_t)

            def finish_mb(mb, pss, h_list=None):
                i = 0 if mb < Q_SIZE // P else (1 if mb < (Q_SIZE + KV_SIZE) // P else 2)
                st = st_pool.tile([P, TC], f32, tag="st", name=f"st{mb}")
                oq = nc.sync if mb % 2 == 0 else nc.scalar
                for h in (h_list if h_list is not None else range(NH)):
                    nc.tensor.matmul(
                        pss[h][:],
                        b_t[:, mb, :],
                        ag[i][:, h * 512:(h + 1) * 512],
                        start=False, stop=True,
                    )
                    nc.vector.tensor_scalar_mul(
                        st[:, h * 512:(h + 1) * 512], pss[h][:],
                        float(1.0 / SP))
                    oq.dma_start(
                        out=y_d[mb, :, h * 512:(h + 1) * 512],
                        in_=st[:, h * 512:(h + 1) * 512],
                    )
                return st

            finish_mb(0, ps0)
            finish_mb(1, ps1)

            # ------------- Phase C: remaining mb chains ---------------------
            # h-sequential: one psum bank per half, half0's B/copy/DMA overlap
            # half1's chain; last mb's final DMAs split to shorten the tail.
            for mb in range(2, MB):
                w_t, w8_t = load_w(mb, nc.scalar)
                i = 0 if mb < Q_SIZE // P else (1 if mb < (Q_SIZE + KV_SIZE) // P else 2)
                st = st_pool.tile([P, TC], f32, tag="st", name=f"st{mb}")
                oq = nc.sync if mb % 2 == 0 else nc.scalar
                pieces = 2 if mb >= MB - 2 else 1
                for h in range(NH):
                    ps = ps_pool.tile([P, 512], f32, tag="ps", name=f"pm{mb}_{h}")
                    for k in range(KB):
                        nc.tensor.matmul(
                            ps[:], w_t[:, k, :],
                            x_res[:, k, h * 512:(h + 1) * 512],
                            start=(k == 0), stop=False,
                        )
                    for jp in range(NP8):
                        j = KB // 2 + jp
                        mv = px1_t[:, j, :, 3 * P:] if h == 0 else px2_t[:, j, :, :]
                        nc.tensor.matmul(
                            ps[:], w8_t[:, jp, :, :], mv,
                            start=False, stop=False, perf_mode=DR,
                        )
                    nc.tensor.matmul(
                        ps[:], b_t[:, mb, :],
                        ag[i][:, h * 512:(h + 1) * 512],
                        start=False, stop=True,
                    )
                    nc.vector.tensor_scalar_mul(
                        st[:, h * 512:(h + 1) * 512], ps[:],
                        float(1.0 / SP))
                    pw = 512 // pieces
                    for pc in range(pieces):
                        lo = h * 512 + pc * pw
                        # final mbs: spread pieces over both rings
                        q = (nc.sync, nc.scalar)[pc % 2] if pieces > 1 else oq
                        q.dma_start(
                            out=y_d[mb, :, lo:lo + pw],
                            in_=st[:, lo:lo + pw],
                        )

    nc.compile()
    return nc


def _get_nc():
    if "nc" not in _CACHE:
        _CACHE["nc"] = _build_nc()
    return _CACHE["nc"]


def _prep_in_maps(x, W, lora_A, lora_B_q, lora_B_k, lora_B_v, scaling, token_to_slot):
    f = np.float32
    bf = ml_dtypes.bfloat16
    x = np.ascontiguousarray(x, dtype=f)
    W = np.ascontiguousarray(W, dtype=f)

    # x shard, moving operand: [c, p, ka, tl]  (h = ka*128 + p, t = c*1024 + tl)
    x_f32 = np.ascontiguousarray(
        x.reshape(NCORES, TC, KA, P).transpose(0, 3, 2, 1))
    x_sh = np.ascontiguousarray(x_f32[:, :, :KB, :]).astype(bf)
    # W stationary: [mb, p, ka, dl]  (d = mb*128 + dl)  -- replicated.
    # bf16 part pre-scaled by SP to match the fp8 psum scale; the last
    # 2*NP8 k-tiles go as fp8(W*1024) DoubleRow pairs.
    w_all = W.reshape(MB, P, KA, P).transpose(0, 3, 2, 1)
    w_t = np.ascontiguousarray(w_all[:, :, :KB, :] * SP).astype(bf)
    # fp8 e4m3 copies for the LoRA down-proj (DoubleRow pairs of k-tiles);
    # pax1 = [A targets | x8 half0], pax2 = [x8 half1]. 1/(SX*SA) descale
    # folds into the gate below.
    SX, SA = np.float32(32.0), np.float32(1024.0)
    f8 = ml_dtypes.float8_e4m3
    w8 = np.ascontiguousarray(
        (w_all[:, :, KB:, :] * SA).astype(f8).reshape(MB, P, NP8, 2, P))
    x8 = (x_f32 * SX).astype(f8).reshape(NCORES, P, JA, 2, TC)
    a_f32 = np.ascontiguousarray(
        np.asarray(lora_A, dtype=f).reshape(S, 3, R, KA, P).transpose(4, 3, 1, 0, 2)
        .reshape(P, KA, 3, S * R))
    a8 = (a_f32 * SA).astype(f8).reshape(P, JA, 2, 3 * S * R)
    pax1 = np.concatenate(
        [np.broadcast_to(a8, (NCORES,) + a8.shape), x8[..., 0:512]], axis=-1)
    pax1 = np.ascontiguousarray(pax1)
    pax2 = np.ascontiguousarray(x8[..., 512:])
    # LoRA B stationary: [(s r), mb, dl] -- replicated
    bq = np.asarray(lora_B_q, dtype=f).transpose(0, 2, 1).reshape(S * R, Q_SIZE)
    bk = np.asarray(lora_B_k, dtype=f).transpose(0, 2, 1).reshape(S * R, KV_SIZE)
    bv = np.asarray(lora_B_v, dtype=f).transpose(0, 2, 1).reshape(S * R, KV_SIZE)
    b_t = np.ascontiguousarray(
        np.concatenate([bq, bk, bv], axis=1).reshape(S * R, MB, P)).astype(bf)
    # routing gate, expanded over ranks: [c, (s r), tl]. The LoRA psum is
    # already SP x true scale (x*32 times A*1024), which matches the main
    # psum scale, so the gate is just the per-slot scaling.
    slot = np.asarray(token_to_slot).reshape(NCORES, TC)
    g = (slot[:, None, :] == np.arange(S, dtype=slot.dtype)[None, :, None])
    g = g.astype(f) * np.asarray(scaling, dtype=f)[None, :, None]
    gate = np.ascontiguousarray(np.repeat(g, R, axis=1))

    in_maps = []
    for c in range(NCORES):
        in_maps.append({
            "x_sh": x_sh[c],
            "w_t": w_t,
            "w8_t": w8,
            "pax1": pax1[c],
            "pax2": pax2[c],
            "b_t": b_t,
            "gate": gate[c],
        })
    return in_maps


def _assemble(results):
    out = np.empty((T, D), dtype=np.float32)
    for c in range(NCORES):
        out[c * TC:(c + 1) * TC, :] = results[c]["y_out"].reshape(D, TC).T
    return out


def _run(inputs, trace=False):
    from concourse.bass_utils import run_bass_kernel_spmd
    nc = _get_nc()
    in_maps = _prep_in_maps(**inputs)
    res = run_bass_kernel_spmd(
        nc, in_maps, core_ids=list(range(NCORES)), trace=trace)
    return res


def kernel(**inputs) -> np.ndarray:
    res = _run(inputs, trace=False)
    return _assemble(res.results)


if __name__ == "__main__":
    rng = np.random.default_rng(0)
    ins = {
        "x": rng.standard_normal((T, HID)).astype(np.float32),
        "W": (rng.standard_normal((D, HID)) * 0.02).astype(np.float32),
        "lora_A": (rng.standard_normal((S, 3, R, HID)) * 0.02).astype(np.float32),
        "lora_B_q": (rng.standard_normal((S, Q_SIZE, R)) * 0.02).astype(np.float32),
        "lora_B_k": (rng.standard_normal((S, KV_SIZE, R)) * 0.02).astype(np.float32),
        "lora_B_v": (rng.standard_normal((S, KV_SIZE, R)) * 0.02).astype(np.float32),
        "scaling": rng.uniform(0.5, 2.0, S).astype(np.float32),
        "token_to_slot": rng.integers(0, S, T).astype(np.int32),
    }
    out = kernel(**ins)
    print("out", out.shape, out.dtype)
